# revision 7
# baseline (speedup 1.0000x reference)
"""Trainium2 Bass kernel for nn_CrossAttention (N=16,Q=4096,C=77,D=512,Dc=768,H=8,S=64).

Sharding: data-parallel over batch N across 8 cores (2 batches/core, no collectives).

Per-chunk structure (CHUNK=512 query rows), all matmul operands bf16:
  q_raw[i,d]    <- gpsimd cast-DMA (f32 dram -> bf16 sbuf), prefetched 2 chunks ahead
  queryT[d,i]   <- DMA XBAR transpose (off the PE engine), prefetched 1 chunk ahead
  qT[s2,hp,i]   <- Wq_pair.T @ queryT          (PE, N=512)
  scoresT[c,i]  <- kT_h.T @ qT_h               (PE, N=512)
  expT[c,h,i]   <- exp(scoresT * 1/sqrt(S))    (Act, scale folded into exp)
  av_nat[i,65]  <- expT_h_ib.T @ [v_h | 1]     (PE, N=65: av cols 0:64, colsum col 64)
  attn_nat      <- av * recip(colsum)          (DVE: strided recip + stride-0 bcast mult)
  attnT[hs,i]   <- DMA XBAR transpose of attn_nat
  out[i,d]      <- attnT.T @ Wo                (PE, one chunk behind, interleaved with
                                                the next chunk's av work)
"""

import sys

if "/opt/trn_rl_repo" not in sys.path:
    sys.path.insert(0, "/opt/trn_rl_repo")

import numpy as np

import concourse.bass as bass
import concourse.tile as tile
from concourse import bacc, mybir
from concourse.bass_utils import run_bass_kernel_spmd

# Problem shapes (hardcoded per spec)
N, Q, C = 16, 4096, 77
D, DC, H, S = 512, 768, 8, 64
HS = H * S  # 512
N_CORES = 8
NB = N // N_CORES  # batches per core = 2
P = 128
CHUNK = 512
N_CHUNKS = Q // CHUNK  # 8
IT = CHUNK // P  # 4 i-tiles per chunk
N_PAIRS = H // 2  # 4
KT_D = D // P  # 4
KT_DC = DC // P  # 6
CPADT = 80  # ctx rows padded to /16 for DMA transpose
VA = S + 1  # 65: v columns + ones column

F32 = mybir.dt.float32
BF16 = mybir.dt.bfloat16


def build_kernel(use_f32r=True, with_bias=True, pools=None):
    nc = bacc.Bacc("TRN2", target_bir_lowering=False, debug=False,
                   num_devices=N_CORES, dynamic_dma_scratch_size=65536)

    query = nc.dram_tensor("query", [NB, Q, D], F32, kind="ExternalInput").ap()
    context = nc.dram_tensor("context", [NB, C, DC], F32, kind="ExternalInput").ap()
    Wq = nc.dram_tensor("Wq", [D, HS], F32, kind="ExternalInput").ap()
    Wk = nc.dram_tensor("Wk", [DC, HS], F32, kind="ExternalInput").ap()
    Wv = nc.dram_tensor("Wv", [DC, HS], F32, kind="ExternalInput").ap()
    Wo = nc.dram_tensor("Wo", [HS, D], F32, kind="ExternalInput").ap()
    bo = nc.dram_tensor("bo", [D], F32, kind="ExternalInput").ap()
    out = nc.dram_tensor("out", [NB, Q, D], F32, kind="ExternalOutput").ap()

    with tile.TileContext(nc) as tc:
        _emit(nc, tc, query, context, Wq, Wk, Wv, Wo, bo, out, with_bias,
              pools or {})
    nc.compile()
    return nc


def _emit(nc, tc, query, context, Wq, Wk, Wv, Wo, bo, out, with_bias, pools):
    from contextlib import ExitStack

    pg = lambda k, d: pools.get(k, d)
    scale = float(S) ** -0.5
    total = NB * N_CHUNKS

    ctx = ExitStack()
    with ctx:
        consts = ctx.enter_context(tc.tile_pool(name="consts", bufs=1))
        wpool = ctx.enter_context(tc.tile_pool(name="weights", bufs=1))
        ctxp = ctx.enter_context(tc.tile_pool(name="ctxphase", bufs=1))
        qin = ctx.enter_context(tc.tile_pool(name="qin", bufs=pg("qin", 3)))
        qtp = ctx.enter_context(tc.tile_pool(name="qtp", bufs=pg("qtp", 2)))
        qtc = ctx.enter_context(tc.tile_pool(name="qtc", bufs=pg("qtc", 2)))
        expp = ctx.enter_context(tc.tile_pool(name="expp", bufs=pg("expp", 2)))
        anp = ctx.enter_context(tc.tile_pool(name="attnat", bufs=pg("anp", 6)))
        rcpp = ctx.enter_context(tc.tile_pool(name="rcp", bufs=pg("rcpp", 4)))
        atp = ctx.enter_context(tc.tile_pool(name="attnT", bufs=pg("atp", 2)))
        outp = ctx.enter_context(tc.tile_pool(name="outp", bufs=pg("outp", 2)))

        ps_qp = ctx.enter_context(tc.tile_pool(name="ps_qp", bufs=pg("qp", 2), space="PSUM"))
        ps_sc = ctx.enter_context(tc.tile_pool(name="ps_sc", bufs=pg("sc", 2), space="PSUM"))
        ps_av = ctx.enter_context(tc.tile_pool(name="ps_av", bufs=pg("av", 2), space="PSUM"))
        ps_o = ctx.enter_context(tc.tile_pool(name="ps_o", bufs=pg("o", 2), space="PSUM"))

        # ---- ctx loads via SP f32 DMA + DVE convert (avoids the SWDGE ring) ----
        ctx_f32, ctx_bf = [], []
        for b in range(NB):
            cf = ctxp.tile([C, DC], F32, tag=f"ctxf{b}", name=f"ctxf{b}")
            cb = ctxp.tile([CPADT, DC], BF16, tag=f"ctxbf{b}", name=f"ctxbf{b}")
            ctx_f32.append(cf)
            ctx_bf.append(cb)
        nc.sync.dma_start(ctx_f32[0][:], context[0])
        nc.sync.dma_start(ctx_f32[1][:], context[1])
        for b in range(NB):
            nc.gpsimd.memset(ctx_bf[b][C:CPADT, :], 0.0)

        # ---- weights via gpsimd cast-DMA (SWDGE), in dependency-critical order ----
        wk_sb = wpool.tile([P, KT_DC, HS], BF16)
        wv_sb = wpool.tile([P, KT_DC, HS], BF16)
        wq_sb = wpool.tile([P, KT_D, HS], BF16)
        wo_sb = wpool.tile([P, KT_D, D], BF16)
        nc.gpsimd.dma_start(wk_sb[:], Wk.rearrange("(kt p) n -> p kt n", p=P))
        nc.gpsimd.dma_start(wv_sb[:], Wv.rearrange("(kt p) n -> p kt n", p=P))
        nc.gpsimd.dma_start(wq_sb[:], Wq.rearrange("(kt p) n -> p kt n", p=P))

        q_raws = [None] * total
        q_raws[0] = qin.tile([P, IT, CHUNK], BF16, tag="q_raw", name="q_raw0")
        nc.gpsimd.dma_start(
            q_raws[0][:], query[0, 0:CHUNK, :].rearrange("(t p) c -> p t c", p=P))

        nc.gpsimd.dma_start(wo_sb[:], Wo.rearrange("(kt p) n -> p kt n", p=P))

        q_raws[1] = qin.tile([P, IT, CHUNK], BF16, tag="q_raw", name="q_raw1")
        nc.gpsimd.dma_start(
            q_raws[1][:], query[0, CHUNK:2 * CHUNK, :].rearrange("(t p) c -> p t c", p=P))

        if with_bias:
            onesrow = consts.tile([1, P], BF16)
            nc.gpsimd.memset(onesrow[:], 1.0)
            bo_sb = consts.tile([1, D], BF16)
            nc.gpsimd.dma_start(bo_sb[:], bo[None, :])

        # ---- ctx convert + DMA transpose ----
        ctxT = []
        for b in range(NB):
            nc.vector.tensor_copy(ctx_bf[b][:C, :], ctx_f32[b][:])
            ct = ctxp.tile([P, KT_DC, CPADT], BF16, tag=f"ctxT{b}", name=f"ctxT{b}")
            nc.sync.dma_start(ct[:], ctx_bf[b][:], transpose=True)
            ctxT.append(ct)

        kT = [None] * NB
        v_aug = [None] * NB

        def emit_ctx_phase(b):
            kT_b = ctxp.tile([P, N_PAIRS, C], BF16, tag=f"kT{b}", name=f"kT{b}")
            va_b = ctxp.tile([C, H, VA], BF16, tag=f"vaug{b}", name=f"vaug{b}")
            nc.gpsimd.memset(va_b[:, :, S:VA], 1.0)
            for hp in range(N_PAIRS):
                pk = ps_sc.tile([P, CHUNK], F32, tag="sc")
                for kt in range(KT_DC):
                    nc.tensor.matmul(
                        pk[:, :C],
                        wk_sb[:, kt, hp * P:(hp + 1) * P],
                        ctxT[b][:, kt, :C],
                        start=(kt == 0), stop=(kt == KT_DC - 1),
                    )
                nc.vector.tensor_copy(kT_b[:, hp, :], pk[:, :C])
            for hp in range(N_PAIRS):
                pv = ps_av.tile([P, CHUNK], F32, tag="av")
                for kt in range(KT_DC):
                    nc.tensor.matmul(
                        pv[:C, :P],
                        ctxT[b][:, kt, :C],
                        wv_sb[:, kt, hp * P:(hp + 1) * P],
                        start=(kt == 0), stop=(kt == KT_DC - 1),
                    )
                nc.vector.tensor_copy(
                    va_b[:, 2 * hp:2 * hp + 2, 0:S],
                    pv[:C, :P].rearrange("c (h s) -> c h s", h=2),
                )
            kT[b] = kT_b
            v_aug[b] = va_b

        emit_ctx_phase(0)

        # queryT for chunk 0 (prologue)
        def emit_qtp(q_raw_t):
            qt = qtp.tile([P, KT_D, CHUNK], BF16, tag="queryT")
            for it in range(IT):
                nc.sync.dma_start(
                    qt[:, :, it * P:(it + 1) * P], q_raw_t[:, it, :],
                    transpose=True)
            return qt

        queryT = [None] * total
        queryT[0] = emit_qtp(q_raws[0])

        # ---- o-proj helpers (one chunk behind, emitted per-it) ----
        oproj_state = {}

        def emit_oproj_it(b, ch, attnT_c, it):
            if it == 0:
                oproj_state["outc"] = outp.tile([P, IT, D], F32, tag="outc", name="outc")
            outc = oproj_state["outc"]
            po = ps_o.tile([P, D], F32, tag="o")
            for kt in range(KT_D):
                nc.tensor.matmul(
                    po[:],
                    attnT_c[:, kt, it * P:(it + 1) * P],
                    wo_sb[:, kt, :],
                    start=(kt == 0),
                    stop=(not with_bias and kt == KT_D - 1),
                )
            if with_bias:
                nc.tensor.matmul(po[:], onesrow[:], bo_sb[:], start=False, stop=True)
            nc.vector.tensor_copy(outc[:, it, :], po[:])
            if it == IT - 1:
                nc.gpsimd.dma_start(
                    out[b, ch * CHUNK:(ch + 1) * CHUNK, :]
                    .rearrange("(t p) c -> p t c", p=P),
                    outc[:],
                )

        pending = None  # (b, ch, attnT_tile)

        # ---- main loop ----
        for step in range(total):
            b, ch = divmod(step, N_CHUNKS)

            # prefetch q_raw two ahead, queryT one ahead
            if step + 2 < total:
                nb_, nch = divmod(step + 2, N_CHUNKS)
                q_raws[step + 2] = qin.tile([P, IT, CHUNK], BF16, tag="q_raw",
                                            name=f"q_raw{step+2}")
                nc.gpsimd.dma_start(
                    q_raws[step + 2][:],
                    query[nb_, nch * CHUNK:(nch + 1) * CHUNK, :]
                    .rearrange("(t p) c -> p t c", p=P))
            queryT_c = queryT[step]

            # q-proj + scores + exp, interleaved
            qT_c = qtc.tile([P, N_PAIRS, CHUNK], BF16, tag="qT")
            ps_list = [None] * (2 * N_PAIRS)

            def emit_qproj(hp):
                pq = ps_qp.tile([P, CHUNK], F32, tag="qp")
                for kt in range(KT_D):
                    nc.tensor.matmul(
                        pq[:],
                        wq_sb[:, kt, hp * P:(hp + 1) * P],
                        queryT_c[:, kt, :],
                        start=(kt == 0), stop=(kt == KT_D - 1),
                    )
                if hp in (0, 1):
                    nc.vector.tensor_copy(qT_c[:, hp, :], pq[:])
                else:
                    nc.scalar.copy(qT_c[:, hp, :], pq[:])

            def emit_scores(hp):
                ps0 = ps_sc.tile([P, CHUNK], F32, tag="sc")
                ps1 = ps_sc.tile([P, CHUNK], F32, tag="sc")
                nc.tensor.matmul(ps0[:C, :], kT[b][0:S, hp, :],
                                 qT_c[0:S, hp, :], start=True, stop=True)
                nc.tensor.matmul(ps1[:C, :], kT[b][S:P, hp, :],
                                 qT_c[S:P, hp, :], start=True, stop=True)
                ps_list[2 * hp] = ps0
                ps_list[2 * hp + 1] = ps1

            expT_c = expp.tile([C, H, CHUNK], BF16, tag="expT")

            def emit_exp(hp):
                for hh in range(2):
                    h = 2 * hp + hh
                    nc.scalar.activation(
                        expT_c[:, h, :], ps_list[2 * hp + hh][:C, :],
                        mybir.ActivationFunctionType.Exp, scale=scale,
                    )

            emit_qproj(0)
            emit_qproj(1)
            emit_scores(0)
            if step + 1 < total:
                queryT[step + 1] = emit_qtp(q_raws[step + 1])
            emit_exp(0)
            emit_qproj(2)
            emit_scores(1)
            emit_exp(1)
            emit_qproj(3)
            emit_scores(2)
            emit_exp(2)
            emit_scores(3)
            emit_exp(3)

            if step == 0:
                # fill chunk-0's exp wait with batch-1 ctx compute
                emit_ctx_phase(1)

            # av + normalize + attnT transpose, interleaved with prev o-proj
            attnT_c = atp.tile([P, KT_D, CHUNK], BF16, tag="attnT")

            def emit_av_block(ib):
                pavA = ps_av.tile([P, 4 * VA], F32, tag="av")
                pavB = ps_av.tile([P, 4 * VA], F32, tag="av")
                for h in range(H):
                    pav = pavA if h < 4 else pavB
                    g = h % 4
                    nc.tensor.matmul(
                        pav[:, g * VA:(g + 1) * VA],
                        expT_c[:, h, ib * P:(ib + 1) * P],
                        v_aug[b][:, h, :],
                        start=True, stop=True,
                    )
                attn_nat = anp.tile([P, HS], BF16, tag="attn_nat")
                for half, pav in ((0, pavA), (1, pavB)):
                    rcp = rcpp.tile([P, 4], F32, tag="rcp")
                    grp = pav[:].rearrange("p (g c) -> p g c", g=4)
                    nc.vector.reciprocal(rcp[:], grp[:, :, S])
                    nc.vector.tensor_tensor(
                        attn_nat[:, half * 256:(half + 1) * 256]
                        .rearrange("p (g c) -> p g c", g=4),
                        grp[:, :, 0:S],
                        rcp[:].unsqueeze(2).to_broadcast((P, 4, S)),
                        mybir.AluOpType.mult,
                    )
                nc.sync.dma_start(
                    attnT_c[:, :, ib * P:(ib + 1) * P], attn_nat[:],
                    transpose=True)

            if pending is not None:
                emit_oproj_it(*pending, 0)
                emit_oproj_it(*pending, 1)
            emit_av_block(0)
            if pending is not None:
                emit_oproj_it(*pending, 2)
            emit_av_block(1)
            if pending is not None:
                emit_oproj_it(*pending, 3)
            emit_av_block(2)
            emit_av_block(3)

            pending = (b, ch, attnT_c)

        for it in range(IT):
            emit_oproj_it(*pending, it)


_CACHE = {}


def _get_nc(use_f32r=True, with_bias=True):
    key = (use_f32r, with_bias)
    if key not in _CACHE:
        _CACHE[key] = build_kernel(use_f32r, with_bias)
    return _CACHE[key]


def kernel(query, context, Wq, Wk, Wv, Wo, bo, _use_f32r=True):
    query = np.ascontiguousarray(np.asarray(query, dtype=np.float32))
    context = np.ascontiguousarray(np.asarray(context, dtype=np.float32))
    Wq = np.asarray(Wq, dtype=np.float32).reshape(D, HS)
    Wk = np.asarray(Wk, dtype=np.float32).reshape(DC, HS)
    Wv = np.asarray(Wv, dtype=np.float32).reshape(DC, HS)
    Wo = np.asarray(Wo, dtype=np.float32).reshape(HS, D)
    bo = np.asarray(bo, dtype=np.float32).reshape(D)

    nc = _get_nc(use_f32r=_use_f32r, with_bias=bool(np.any(bo)))
    in_maps = []
    for c in range(N_CORES):
        sl = slice(c * NB, (c + 1) * NB)
        in_maps.append({
            "query": np.ascontiguousarray(query[sl]),
            "context": np.ascontiguousarray(context[sl]),
            "Wq": Wq, "Wk": Wk, "Wv": Wv, "Wo": Wo, "bo": bo,
        })
    res = run_bass_kernel_spmd(nc, in_maps, core_ids=list(range(N_CORES)))
    return np.concatenate([res.results[c]["out"] for c in range(N_CORES)], axis=0)


# revision 8
# speedup vs baseline: 1.1797x; 1.1797x over previous
"""Trainium2 Bass kernel for nn_CrossAttention (N=16,Q=4096,C=77,D=512,Dc=768,H=8,S=64).

Sharding: data-parallel over batch N across 8 cores (2 batches/core, no collectives).

Per-chunk structure (CHUNK=512 query rows), all matmul operands bf16:
  q_raw[i,d]    <- gpsimd cast-DMA (f32 dram -> bf16 sbuf), prefetched 2 chunks ahead
  queryT[d,i]   <- DMA XBAR transpose (off the PE engine), prefetched 1 chunk ahead
  qT[s2,hp,i]   <- Wq_pair.T @ queryT          (PE, N=512)
  scoresT[c,i]  <- kT_h.T @ qT_h               (PE, N=512)
  expT[c,h,i]   <- exp(scoresT * 1/sqrt(S))    (Act, scale folded into exp)
  av_nat[i,65]  <- expT_h_ib.T @ [v_h | 1]     (PE, N=65: av cols 0:64, colsum col 64)
  attn_nat      <- av * recip(colsum)          (DVE: strided recip + stride-0 bcast mult)
  attnT[hs,i]   <- DMA XBAR transpose of attn_nat
  out[i,d]      <- attnT.T @ Wo                (PE, one chunk behind, interleaved with
                                                the next chunk's av work)
"""

import sys

if "/opt/trn_rl_repo" not in sys.path:
    sys.path.insert(0, "/opt/trn_rl_repo")

import numpy as np

import concourse.bass as bass
import concourse.tile as tile
from concourse import bacc, mybir
from concourse.bass_utils import run_bass_kernel_spmd

# Problem shapes (hardcoded per spec)
N, Q, C = 16, 4096, 77
D, DC, H, S = 512, 768, 8, 64
HS = H * S  # 512
N_CORES = 8
NB = N // N_CORES  # batches per core = 2
P = 128
CHUNK = 512
N_CHUNKS = Q // CHUNK  # 8
IT = CHUNK // P  # 4 i-tiles per chunk
N_PAIRS = H // 2  # 4
KT_D = D // P  # 4
KT_DC = DC // P  # 6
CPADT = 80  # ctx rows padded to /16 for DMA transpose
VA = S + 1  # 65: v columns + ones column

F32 = mybir.dt.float32
BF16 = mybir.dt.bfloat16


def build_kernel(use_f32r=True, with_bias=True, pools=None):
    nc = bacc.Bacc("TRN2", target_bir_lowering=False, debug=False,
                   num_devices=N_CORES, dynamic_dma_scratch_size=65536)

    query = nc.dram_tensor("query", [NB, Q, D], F32, kind="ExternalInput").ap()
    context = nc.dram_tensor("context", [NB, C, DC], F32, kind="ExternalInput").ap()
    Wq = nc.dram_tensor("Wq", [D, HS], F32, kind="ExternalInput").ap()
    Wk = nc.dram_tensor("Wk", [DC, HS], F32, kind="ExternalInput").ap()
    Wv = nc.dram_tensor("Wv", [DC, HS], F32, kind="ExternalInput").ap()
    Wo = nc.dram_tensor("Wo", [HS, D], F32, kind="ExternalInput").ap()
    bo = nc.dram_tensor("bo", [D], F32, kind="ExternalInput").ap()
    out = nc.dram_tensor("out", [NB, Q, D], F32, kind="ExternalOutput").ap()

    with tile.TileContext(nc) as tc:
        _emit(nc, tc, query, context, Wq, Wk, Wv, Wo, bo, out, with_bias,
              pools or {})
    nc.compile()
    return nc


def _emit(nc, tc, query, context, Wq, Wk, Wv, Wo, bo, out, with_bias, pools):
    from contextlib import ExitStack

    pg = lambda k, d: pools.get(k, d)
    scale = float(S) ** -0.5
    total = NB * N_CHUNKS

    ctx = ExitStack()
    with ctx:
        consts = ctx.enter_context(tc.tile_pool(name="consts", bufs=1))
        wpool = ctx.enter_context(tc.tile_pool(name="weights", bufs=1))
        ctxp = ctx.enter_context(tc.tile_pool(name="ctxphase", bufs=1))
        qin = ctx.enter_context(tc.tile_pool(name="qin", bufs=pg("qin", 3)))
        qtp = ctx.enter_context(tc.tile_pool(name="qtp", bufs=pg("qtp", 2)))
        qtc = ctx.enter_context(tc.tile_pool(name="qtc", bufs=pg("qtc", 2)))
        expp = ctx.enter_context(tc.tile_pool(name="expp", bufs=pg("expp", 2)))
        anp = ctx.enter_context(tc.tile_pool(name="attnat", bufs=pg("anp", 6)))
        rcpp = ctx.enter_context(tc.tile_pool(name="rcp", bufs=pg("rcpp", 4)))
        atp = ctx.enter_context(tc.tile_pool(name="attnT", bufs=pg("atp", 2)))
        outp = ctx.enter_context(tc.tile_pool(name="outp", bufs=pg("outp", 2)))

        ps_qp = ctx.enter_context(tc.tile_pool(name="ps_qp", bufs=pg("qp", 2), space="PSUM"))
        ps_sc = ctx.enter_context(tc.tile_pool(name="ps_sc", bufs=pg("sc", 2), space="PSUM"))
        ps_av = ctx.enter_context(tc.tile_pool(name="ps_av", bufs=pg("av", 2), space="PSUM"))
        ps_o = ctx.enter_context(tc.tile_pool(name="ps_o", bufs=pg("o", 2), space="PSUM"))

        # ---- ctx loads via SP f32 DMA + DVE convert (avoids the SWDGE ring) ----
        ctx_f32, ctx_bf = [], []
        for b in range(NB):
            cf = ctxp.tile([C, DC], F32, tag=f"ctxf{b}", name=f"ctxf{b}")
            cb = ctxp.tile([CPADT, DC], BF16, tag=f"ctxbf{b}", name=f"ctxbf{b}")
            ctx_f32.append(cf)
            ctx_bf.append(cb)
        nc.sync.dma_start(ctx_f32[0][:], context[0])
        nc.sync.dma_start(ctx_f32[1][:], context[1])
        for b in range(NB):
            nc.gpsimd.memset(ctx_bf[b][C:CPADT, :], 0.0)

        # ---- weights via gpsimd cast-DMA (SWDGE), in dependency-critical order ----
        wk_sb = wpool.tile([P, KT_DC, HS], BF16)
        wv_sb = wpool.tile([P, KT_DC, HS], BF16)
        wq_sb = wpool.tile([P, KT_D, HS], BF16)
        wo_sb = wpool.tile([P, KT_D, D], BF16)
        nc.gpsimd.dma_start(wk_sb[:], Wk.rearrange("(kt p) n -> p kt n", p=P))
        nc.gpsimd.dma_start(wv_sb[:], Wv.rearrange("(kt p) n -> p kt n", p=P))
        nc.gpsimd.dma_start(wq_sb[:], Wq.rearrange("(kt p) n -> p kt n", p=P))

        q_raws = [None] * total
        q_raws[0] = qin.tile([P, IT, CHUNK], BF16, tag="q_raw", name="q_raw0")
        nc.gpsimd.dma_start(
            q_raws[0][:], query[0, 0:CHUNK, :].rearrange("(t p) c -> p t c", p=P))

        nc.gpsimd.dma_start(wo_sb[:], Wo.rearrange("(kt p) n -> p kt n", p=P))

        q_raws[1] = qin.tile([P, IT, CHUNK], BF16, tag="q_raw", name="q_raw1")
        nc.gpsimd.dma_start(
            q_raws[1][:], query[0, CHUNK:2 * CHUNK, :].rearrange("(t p) c -> p t c", p=P))

        if with_bias:
            onesrow = consts.tile([1, P], BF16)
            nc.gpsimd.memset(onesrow[:], 1.0)
            bo_sb = consts.tile([1, D], BF16)
            nc.gpsimd.dma_start(bo_sb[:], bo[None, :])

        # ---- ctx convert + DMA transpose ----
        ctxT = []
        for b in range(NB):
            nc.vector.tensor_copy(ctx_bf[b][:C, :], ctx_f32[b][:])
            ct = ctxp.tile([P, KT_DC, CPADT], BF16, tag=f"ctxT{b}", name=f"ctxT{b}")
            nc.sync.dma_start(ct[:], ctx_bf[b][:], transpose=True)
            ctxT.append(ct)

        kT = [None] * NB
        v_aug = [None] * NB

        def emit_ctx_phase(b):
            kT_b = ctxp.tile([P, N_PAIRS, C], BF16, tag=f"kT{b}", name=f"kT{b}")
            va_b = ctxp.tile([C, H, VA], BF16, tag=f"vaug{b}", name=f"vaug{b}")
            nc.gpsimd.memset(va_b[:, :, S:VA], 1.0)
            for hp in range(N_PAIRS):
                pk = ps_sc.tile([P, CHUNK], F32, tag="sc")
                for kt in range(KT_DC):
                    nc.tensor.matmul(
                        pk[:, :C],
                        wk_sb[:, kt, hp * P:(hp + 1) * P],
                        ctxT[b][:, kt, :C],
                        start=(kt == 0), stop=(kt == KT_DC - 1),
                    )
                nc.vector.tensor_copy(kT_b[:, hp, :], pk[:, :C])
            for hp in range(N_PAIRS):
                pv = ps_av.tile([P, CHUNK], F32, tag="av")
                for kt in range(KT_DC):
                    nc.tensor.matmul(
                        pv[:C, :P],
                        ctxT[b][:, kt, :C],
                        wv_sb[:, kt, hp * P:(hp + 1) * P],
                        start=(kt == 0), stop=(kt == KT_DC - 1),
                    )
                nc.vector.tensor_copy(
                    va_b[:, 2 * hp:2 * hp + 2, 0:S],
                    pv[:C, :P].rearrange("c (h s) -> c h s", h=2),
                )
            kT[b] = kT_b
            v_aug[b] = va_b

        emit_ctx_phase(0)

        # queryT for chunk 0 (prologue)
        def emit_qtp(q_raw_t):
            qt = qtp.tile([P, KT_D, CHUNK], BF16, tag="queryT")
            for it in range(IT):
                nc.sync.dma_start(
                    qt[:, :, it * P:(it + 1) * P], q_raw_t[:, it, :],
                    transpose=True)
            return qt

        queryT = [None] * total
        queryT[0] = emit_qtp(q_raws[0])

        # ---- o-proj helpers (one chunk behind, emitted per-it) ----
        oproj_state = {}

        def emit_oproj_it(b, ch, attnT_c, it):
            if it == 0:
                oproj_state["outc"] = outp.tile([P, IT, D], F32, tag="outc", name="outc")
            outc = oproj_state["outc"]
            po = ps_o.tile([P, D], F32, tag="o")
            for kt in range(KT_D):
                nc.tensor.matmul(
                    po[:],
                    attnT_c[:, kt, it * P:(it + 1) * P],
                    wo_sb[:, kt, :],
                    start=(kt == 0),
                    stop=(not with_bias and kt == KT_D - 1),
                )
            if with_bias:
                nc.tensor.matmul(po[:], onesrow[:], bo_sb[:], start=False, stop=True)
            nc.vector.tensor_copy(outc[:, it, :], po[:])
            if it == IT - 1:
                nc.scalar.dma_start(
                    out[b, ch * CHUNK:(ch + 1) * CHUNK, :]
                    .rearrange("(t p) c -> p t c", p=P),
                    outc[:],
                )

        pending = None  # (b, ch, attnT_tile)

        # ---- main loop ----
        for step in range(total):
            b, ch = divmod(step, N_CHUNKS)

            # prefetch q_raw two ahead, queryT one ahead
            if step + 2 < total:
                nb_, nch = divmod(step + 2, N_CHUNKS)
                q_raws[step + 2] = qin.tile([P, IT, CHUNK], BF16, tag="q_raw",
                                            name=f"q_raw{step+2}")
                nc.gpsimd.dma_start(
                    q_raws[step + 2][:],
                    query[nb_, nch * CHUNK:(nch + 1) * CHUNK, :]
                    .rearrange("(t p) c -> p t c", p=P))
            queryT_c = queryT[step]

            # q-proj + scores + exp, interleaved
            qT_c = qtc.tile([P, N_PAIRS, CHUNK], BF16, tag="qT")
            ps_list = [None] * (2 * N_PAIRS)

            def emit_qproj(hp):
                pq = ps_qp.tile([P, CHUNK], F32, tag="qp")
                for kt in range(KT_D):
                    nc.tensor.matmul(
                        pq[:],
                        wq_sb[:, kt, hp * P:(hp + 1) * P],
                        queryT_c[:, kt, :],
                        start=(kt == 0), stop=(kt == KT_D - 1),
                    )
                if hp in (0, 1):
                    nc.vector.tensor_copy(qT_c[:, hp, :], pq[:])
                else:
                    nc.scalar.copy(qT_c[:, hp, :], pq[:])

            def emit_scores(hp):
                ps0 = ps_sc.tile([P, CHUNK], F32, tag="sc")
                ps1 = ps_sc.tile([P, CHUNK], F32, tag="sc")
                nc.tensor.matmul(ps0[:C, :], kT[b][0:S, hp, :],
                                 qT_c[0:S, hp, :], start=True, stop=True)
                nc.tensor.matmul(ps1[:C, :], kT[b][S:P, hp, :],
                                 qT_c[S:P, hp, :], start=True, stop=True)
                ps_list[2 * hp] = ps0
                ps_list[2 * hp + 1] = ps1

            expT_c = expp.tile([C, H, CHUNK], BF16, tag="expT")

            def emit_exp(hp):
                for hh in range(2):
                    h = 2 * hp + hh
                    nc.scalar.activation(
                        expT_c[:, h, :], ps_list[2 * hp + hh][:C, :],
                        mybir.ActivationFunctionType.Exp, scale=scale,
                    )

            emit_qproj(0)
            emit_qproj(1)
            emit_scores(0)
            if step + 1 < total:
                queryT[step + 1] = emit_qtp(q_raws[step + 1])
            emit_exp(0)
            emit_qproj(2)
            emit_scores(1)
            emit_exp(1)
            emit_qproj(3)
            emit_scores(2)
            emit_exp(2)
            emit_scores(3)
            emit_exp(3)

            if step == 0:
                # fill chunk-0's exp wait with batch-1 ctx compute
                emit_ctx_phase(1)

            # av + normalize + attnT transpose, interleaved with prev o-proj
            attnT_c = atp.tile([P, KT_D, CHUNK], BF16, tag="attnT")

            def emit_av_block(ib):
                pavA = ps_av.tile([P, 4 * VA], F32, tag="av")
                pavB = ps_av.tile([P, 4 * VA], F32, tag="av")
                for h in range(H):
                    pav = pavA if h < 4 else pavB
                    g = h % 4
                    nc.tensor.matmul(
                        pav[:, g * VA:(g + 1) * VA],
                        expT_c[:, h, ib * P:(ib + 1) * P],
                        v_aug[b][:, h, :],
                        start=True, stop=True,
                    )
                attn_nat = anp.tile([P, HS], BF16, tag="attn_nat")
                for half, pav in ((0, pavA), (1, pavB)):
                    rcp = rcpp.tile([P, 4], F32, tag="rcp")
                    grp = pav[:].rearrange("p (g c) -> p g c", g=4)
                    nc.vector.reciprocal(rcp[:], grp[:, :, S])
                    nc.vector.tensor_tensor(
                        attn_nat[:, half * 256:(half + 1) * 256]
                        .rearrange("p (g c) -> p g c", g=4),
                        grp[:, :, 0:S],
                        rcp[:].unsqueeze(2).to_broadcast((P, 4, S)),
                        mybir.AluOpType.mult,
                    )
                nc.sync.dma_start(
                    attnT_c[:, :, ib * P:(ib + 1) * P], attn_nat[:],
                    transpose=True)

            if pending is not None:
                emit_oproj_it(*pending, 0)
                emit_oproj_it(*pending, 1)
            emit_av_block(0)
            if pending is not None:
                emit_oproj_it(*pending, 2)
            emit_av_block(1)
            if pending is not None:
                emit_oproj_it(*pending, 3)
            emit_av_block(2)
            emit_av_block(3)

            pending = (b, ch, attnT_c)

        for it in range(IT):
            emit_oproj_it(*pending, it)


_CACHE = {}


def _get_nc(use_f32r=True, with_bias=True):
    key = (use_f32r, with_bias)
    if key not in _CACHE:
        _CACHE[key] = build_kernel(use_f32r, with_bias)
    return _CACHE[key]


def kernel(query, context, Wq, Wk, Wv, Wo, bo, _use_f32r=True):
    query = np.ascontiguousarray(np.asarray(query, dtype=np.float32))
    context = np.ascontiguousarray(np.asarray(context, dtype=np.float32))
    Wq = np.asarray(Wq, dtype=np.float32).reshape(D, HS)
    Wk = np.asarray(Wk, dtype=np.float32).reshape(DC, HS)
    Wv = np.asarray(Wv, dtype=np.float32).reshape(DC, HS)
    Wo = np.asarray(Wo, dtype=np.float32).reshape(HS, D)
    bo = np.asarray(bo, dtype=np.float32).reshape(D)

    nc = _get_nc(use_f32r=_use_f32r, with_bias=bool(np.any(bo)))
    in_maps = []
    for c in range(N_CORES):
        sl = slice(c * NB, (c + 1) * NB)
        in_maps.append({
            "query": np.ascontiguousarray(query[sl]),
            "context": np.ascontiguousarray(context[sl]),
            "Wq": Wq, "Wk": Wk, "Wv": Wv, "Wo": Wo, "bo": bo,
        })
    res = run_bass_kernel_spmd(nc, in_maps, core_ids=list(range(N_CORES)))
    return np.concatenate([res.results[c]["out"] for c in range(N_CORES)], axis=0)


# revision 9
# speedup vs baseline: 1.3217x; 1.1204x over previous
"""Trainium2 Bass kernel for nn_CrossAttention (N=16,Q=4096,C=77,D=512,Dc=768,H=8,S=64).

Sharding: data-parallel over batch N across 8 cores (2 batches/core, no collectives).

Per-chunk structure (CHUNK=512 query rows), all matmul operands bf16:
  q_raw[i,d]    <- gpsimd cast-DMA (f32 dram -> bf16 sbuf), prefetched 2 chunks ahead
  queryT[d,i]   <- DMA XBAR transpose (off the PE engine), prefetched 1 chunk ahead
  qT[s2,hp,i]   <- Wq_pair.T @ queryT          (PE, N=512)
  scoresT[c,i]  <- kT_h.T @ qT_h               (PE, N=512)
  expT[c,h,i]   <- exp(scoresT * 1/sqrt(S))    (Act, scale folded into exp)
  av_nat[i,65]  <- expT_h_ib.T @ [v_h | 1]     (PE, N=65: av cols 0:64, colsum col 64)
  attn_nat      <- av * recip(colsum)          (DVE: strided recip + stride-0 bcast mult)
  attnT[hs,i]   <- DMA XBAR transpose of attn_nat
  out[i,d]      <- attnT.T @ Wo                (PE, one chunk behind, interleaved with
                                                the next chunk's av work)
"""

import sys

if "/opt/trn_rl_repo" not in sys.path:
    sys.path.insert(0, "/opt/trn_rl_repo")

import numpy as np

import concourse.bass as bass
import concourse.tile as tile
from concourse import bacc, mybir
from concourse.bass_utils import run_bass_kernel_spmd

# Problem shapes (hardcoded per spec)
N, Q, C = 16, 4096, 77
D, DC, H, S = 512, 768, 8, 64
HS = H * S  # 512
N_CORES = 8
NB = N // N_CORES  # batches per core = 2
P = 128
CHUNK = 512
N_CHUNKS = Q // CHUNK  # 8
IT = CHUNK // P  # 4 i-tiles per chunk
N_PAIRS = H // 2  # 4
KT_D = D // P  # 4
KT_DC = DC // P  # 6
CPADT = 80  # ctx rows padded to /16 for DMA transpose
VA = S + 1  # 65: v columns + ones column

F32 = mybir.dt.float32
BF16 = mybir.dt.bfloat16


def build_kernel(use_f32r=True, with_bias=True, pools=None):
    nc = bacc.Bacc("TRN2", target_bir_lowering=False, debug=False,
                   num_devices=N_CORES, dynamic_dma_scratch_size=65536)

    query = nc.dram_tensor("query", [NB, Q, D], F32, kind="ExternalInput").ap()
    context = nc.dram_tensor("context", [NB, C, DC], F32, kind="ExternalInput").ap()
    Wq = nc.dram_tensor("Wq", [D, HS], F32, kind="ExternalInput").ap()
    Wk = nc.dram_tensor("Wk", [DC, HS], F32, kind="ExternalInput").ap()
    Wv = nc.dram_tensor("Wv", [DC, HS], F32, kind="ExternalInput").ap()
    Wo = nc.dram_tensor("Wo", [HS, D], F32, kind="ExternalInput").ap()
    bo = nc.dram_tensor("bo", [D], F32, kind="ExternalInput").ap()
    out = nc.dram_tensor("out", [NB, Q, D], F32, kind="ExternalOutput").ap()

    with tile.TileContext(nc) as tc:
        _emit(nc, tc, query, context, Wq, Wk, Wv, Wo, bo, out, with_bias,
              pools or {})
    nc.compile()
    return nc


def _emit(nc, tc, query, context, Wq, Wk, Wv, Wo, bo, out, with_bias, pools):
    from contextlib import ExitStack

    pg = lambda k, d: pools.get(k, d)
    scale = float(S) ** -0.5
    total = NB * N_CHUNKS

    ctx = ExitStack()
    with ctx:
        consts = ctx.enter_context(tc.tile_pool(name="consts", bufs=1))
        wpool = ctx.enter_context(tc.tile_pool(name="weights", bufs=1))
        ctxp = ctx.enter_context(tc.tile_pool(name="ctxphase", bufs=1))
        qin = ctx.enter_context(tc.tile_pool(name="qin", bufs=pg("qin", 3)))
        qtp = ctx.enter_context(tc.tile_pool(name="qtp", bufs=pg("qtp", 2)))
        qtc = ctx.enter_context(tc.tile_pool(name="qtc", bufs=pg("qtc", 2)))
        expp = ctx.enter_context(tc.tile_pool(name="expp", bufs=pg("expp", 2)))
        anp = ctx.enter_context(tc.tile_pool(name="attnat", bufs=pg("anp", 6)))
        rcpp = ctx.enter_context(tc.tile_pool(name="rcp", bufs=pg("rcpp", 4)))
        atp = ctx.enter_context(tc.tile_pool(name="attnT", bufs=pg("atp", 2)))
        outp = ctx.enter_context(tc.tile_pool(name="outp", bufs=pg("outp", 2)))

        ps_qp = ctx.enter_context(tc.tile_pool(name="ps_qp", bufs=pg("qp", 2), space="PSUM"))
        ps_sc = ctx.enter_context(tc.tile_pool(name="ps_sc", bufs=pg("sc", 2), space="PSUM"))
        ps_av = ctx.enter_context(tc.tile_pool(name="ps_av", bufs=pg("av", 2), space="PSUM"))
        ps_o = ctx.enter_context(tc.tile_pool(name="ps_o", bufs=pg("o", 2), space="PSUM"))

        # ---- ctx loads via SP f32 DMA + DVE convert (avoids the SWDGE ring) ----
        ctx_f32, ctx_bf = [], []
        for b in range(NB):
            cf = ctxp.tile([C, DC], F32, tag=f"ctxf{b}", name=f"ctxf{b}")
            cb = ctxp.tile([CPADT, DC], BF16, tag=f"ctxbf{b}", name=f"ctxbf{b}")
            ctx_f32.append(cf)
            ctx_bf.append(cb)
        nc.sync.dma_start(ctx_f32[0][:], context[0])
        nc.sync.dma_start(ctx_f32[1][:], context[1])
        for b in range(NB):
            nc.gpsimd.memset(ctx_bf[b][C:CPADT, :], 0.0)

        # ---- weights via gpsimd cast-DMA (SWDGE), in dependency-critical order ----
        wk_sb = wpool.tile([P, KT_DC, HS], BF16)
        wv_sb = wpool.tile([P, KT_DC, HS], BF16)
        wq_sb = wpool.tile([P, KT_D, HS], BF16)
        wo_sb = wpool.tile([P, KT_D, D], BF16)
        nc.gpsimd.dma_start(wk_sb[:], Wk.rearrange("(kt p) n -> p kt n", p=P))
        nc.gpsimd.dma_start(wv_sb[:], Wv.rearrange("(kt p) n -> p kt n", p=P))
        nc.gpsimd.dma_start(wq_sb[:], Wq.rearrange("(kt p) n -> p kt n", p=P))

        q_raws = [None] * total
        q_raws[0] = qin.tile([P, IT, CHUNK], BF16, tag="q_raw", name="q_raw0")
        nc.gpsimd.dma_start(
            q_raws[0][:], query[0, 0:CHUNK, :].rearrange("(t p) c -> p t c", p=P))

        nc.gpsimd.dma_start(wo_sb[:], Wo.rearrange("(kt p) n -> p kt n", p=P))

        q_raws[1] = qin.tile([P, IT, CHUNK], BF16, tag="q_raw", name="q_raw1")
        nc.gpsimd.dma_start(
            q_raws[1][:], query[0, CHUNK:2 * CHUNK, :].rearrange("(t p) c -> p t c", p=P))

        if with_bias:
            onesrow = consts.tile([1, P], BF16)
            nc.gpsimd.memset(onesrow[:], 1.0)
            bo_sb = consts.tile([1, D], BF16)
            nc.gpsimd.dma_start(bo_sb[:], bo[None, :])

        # ---- ctx convert + DMA transpose ----
        ctxT = []
        for b in range(NB):
            nc.vector.tensor_copy(ctx_bf[b][:C, :], ctx_f32[b][:])
            ct = ctxp.tile([P, KT_DC, CPADT], BF16, tag=f"ctxT{b}", name=f"ctxT{b}")
            nc.sync.dma_start(ct[:], ctx_bf[b][:], transpose=True)
            ctxT.append(ct)

        kT = [None] * NB
        v_aug = [None] * NB

        def emit_ctx_phase(b):
            kT_b = ctxp.tile([P, N_PAIRS, C], BF16, tag=f"kT{b}", name=f"kT{b}")
            va_b = ctxp.tile([C, H, VA], BF16, tag=f"vaug{b}", name=f"vaug{b}")
            nc.gpsimd.memset(va_b[:, :, S:VA], 1.0)
            for hp in range(N_PAIRS):
                pk = ps_sc.tile([P, CHUNK], F32, tag="sc")
                for kt in range(KT_DC):
                    nc.tensor.matmul(
                        pk[:, :C],
                        wk_sb[:, kt, hp * P:(hp + 1) * P],
                        ctxT[b][:, kt, :C],
                        start=(kt == 0), stop=(kt == KT_DC - 1),
                    )
                nc.vector.tensor_copy(kT_b[:, hp, :], pk[:, :C])
            for hp in range(N_PAIRS):
                pv = ps_av.tile([P, CHUNK], F32, tag="av")
                for kt in range(KT_DC):
                    nc.tensor.matmul(
                        pv[:C, :P],
                        ctxT[b][:, kt, :C],
                        wv_sb[:, kt, hp * P:(hp + 1) * P],
                        start=(kt == 0), stop=(kt == KT_DC - 1),
                    )
                nc.vector.tensor_copy(
                    va_b[:, 2 * hp:2 * hp + 2, 0:S],
                    pv[:C, :P].rearrange("c (h s) -> c h s", h=2),
                )
            kT[b] = kT_b
            v_aug[b] = va_b

        emit_ctx_phase(0)

        # queryT for chunk 0 (prologue)
        def emit_qtp(q_raw_t):
            qt = qtp.tile([P, KT_D, CHUNK], BF16, tag="queryT")
            for it in range(IT):
                nc.sync.dma_start(
                    qt[:, :, it * P:(it + 1) * P], q_raw_t[:, it, :],
                    transpose=True)
            return qt

        queryT = [None] * total
        queryT[0] = emit_qtp(q_raws[0])

        # ---- o-proj helpers (one chunk behind, emitted per-it) ----
        oproj_state = {}

        def emit_oproj_it(b, ch, attnT_c, it):
            if it == 0:
                oproj_state["outc"] = outp.tile([P, IT, D], F32, tag="outc", name="outc")
            outc = oproj_state["outc"]
            po = ps_o.tile([P, D], F32, tag="o")
            for kt in range(KT_D):
                nc.tensor.matmul(
                    po[:],
                    attnT_c[:, kt, it * P:(it + 1) * P],
                    wo_sb[:, kt, :],
                    start=(kt == 0),
                    stop=(not with_bias and kt == KT_D - 1),
                )
            if with_bias:
                nc.tensor.matmul(po[:], onesrow[:], bo_sb[:], start=False, stop=True)
            nc.vector.tensor_copy(outc[:, it, :], po[:])

        pending = None  # (b, ch, attnT_tile)

        # ---- main loop ----
        for step in range(total):
            b, ch = divmod(step, N_CHUNKS)

            # prefetch q_raw two ahead, queryT one ahead
            if step + 2 < total:
                nb_, nch = divmod(step + 2, N_CHUNKS)
                q_raws[step + 2] = qin.tile([P, IT, CHUNK], BF16, tag="q_raw",
                                            name=f"q_raw{step+2}")
                nc.gpsimd.dma_start(
                    q_raws[step + 2][:],
                    query[nb_, nch * CHUNK:(nch + 1) * CHUNK, :]
                    .rearrange("(t p) c -> p t c", p=P))
            queryT_c = queryT[step]

            # q-proj + scores + exp, interleaved
            qT_c = qtc.tile([P, N_PAIRS, CHUNK], BF16, tag="qT")
            ps_list = [None] * (2 * N_PAIRS)

            def emit_qproj(hp):
                pq = ps_qp.tile([P, CHUNK], F32, tag="qp")
                for kt in range(KT_D):
                    nc.tensor.matmul(
                        pq[:],
                        wq_sb[:, kt, hp * P:(hp + 1) * P],
                        queryT_c[:, kt, :],
                        start=(kt == 0), stop=(kt == KT_D - 1),
                    )
                if hp in (0, 1):
                    nc.vector.tensor_copy(qT_c[:, hp, :], pq[:])
                else:
                    nc.scalar.copy(qT_c[:, hp, :], pq[:])

            def emit_scores(hp):
                ps0 = ps_sc.tile([P, CHUNK], F32, tag="sc")
                ps1 = ps_sc.tile([P, CHUNK], F32, tag="sc")
                nc.tensor.matmul(ps0[:C, :], kT[b][0:S, hp, :],
                                 qT_c[0:S, hp, :], start=True, stop=True)
                nc.tensor.matmul(ps1[:C, :], kT[b][S:P, hp, :],
                                 qT_c[S:P, hp, :], start=True, stop=True)
                ps_list[2 * hp] = ps0
                ps_list[2 * hp + 1] = ps1

            expT_c = expp.tile([C, H, CHUNK], BF16, tag="expT")

            def emit_exp(hp):
                for hh in range(2):
                    h = 2 * hp + hh
                    nc.scalar.activation(
                        expT_c[:, h, :], ps_list[2 * hp + hh][:C, :],
                        mybir.ActivationFunctionType.Exp, scale=scale,
                    )

            emit_qproj(0)
            emit_qproj(1)
            emit_scores(0)
            if step + 1 < total:
                queryT[step + 1] = emit_qtp(q_raws[step + 1])
            emit_exp(0)
            emit_qproj(2)
            emit_scores(1)
            emit_exp(1)
            emit_qproj(3)
            emit_scores(2)
            emit_exp(2)
            emit_scores(3)
            emit_exp(3)

            if step == 0:
                # fill chunk-0's exp wait with batch-1 ctx compute
                emit_ctx_phase(1)

            # av + normalize + attnT transpose, interleaved with prev o-proj
            attnT_c = atp.tile([P, KT_D, CHUNK], BF16, tag="attnT")

            def emit_av_block(ib):
                pavA = ps_av.tile([P, 4 * VA], F32, tag="av")
                pavB = ps_av.tile([P, 4 * VA], F32, tag="av")
                for h in range(H):
                    pav = pavA if h < 4 else pavB
                    g = h % 4
                    nc.tensor.matmul(
                        pav[:, g * VA:(g + 1) * VA],
                        expT_c[:, h, ib * P:(ib + 1) * P],
                        v_aug[b][:, h, :],
                        start=True, stop=True,
                    )
                attn_nat = anp.tile([P, HS], BF16, tag="attn_nat")
                for half, pav in ((0, pavA), (1, pavB)):
                    rcp = rcpp.tile([P, 4], F32, tag="rcp")
                    grp = pav[:].rearrange("p (g c) -> p g c", g=4)
                    nc.vector.reciprocal(rcp[:], grp[:, :, S])
                    nc.vector.tensor_tensor(
                        attn_nat[:, half * 256:(half + 1) * 256]
                        .rearrange("p (g c) -> p g c", g=4),
                        grp[:, :, 0:S],
                        rcp[:].unsqueeze(2).to_broadcast((P, 4, S)),
                        mybir.AluOpType.mult,
                    )
                nc.sync.dma_start(
                    attnT_c[:, :, ib * P:(ib + 1) * P], attn_nat[:],
                    transpose=True)

            if pending is not None:
                emit_oproj_it(*pending, 0)
                emit_oproj_it(*pending, 1)
            emit_av_block(0)
            if pending is not None:
                emit_oproj_it(*pending, 2)
            emit_av_block(1)
            if pending is not None:
                emit_oproj_it(*pending, 3)
            emit_av_block(2)
            emit_av_block(3)

            if pending is not None:
                pb, pch, _ = pending
                nc.sync.dma_start(
                    out[pb, pch * CHUNK:(pch + 1) * CHUNK, :]
                    .rearrange("(t p) c -> p t c", p=P),
                    oproj_state["outc"][:],
                )
            pending = (b, ch, attnT_c)

        for it in range(IT):
            emit_oproj_it(*pending, it)
        pb, pch, _ = pending
        nc.sync.dma_start(
            out[pb, pch * CHUNK:(pch + 1) * CHUNK, :]
            .rearrange("(t p) c -> p t c", p=P),
            oproj_state["outc"][:],
        )


_CACHE = {}


def _get_nc(use_f32r=True, with_bias=True):
    key = (use_f32r, with_bias)
    if key not in _CACHE:
        _CACHE[key] = build_kernel(use_f32r, with_bias)
    return _CACHE[key]


def kernel(query, context, Wq, Wk, Wv, Wo, bo, _use_f32r=True):
    query = np.ascontiguousarray(np.asarray(query, dtype=np.float32))
    context = np.ascontiguousarray(np.asarray(context, dtype=np.float32))
    Wq = np.asarray(Wq, dtype=np.float32).reshape(D, HS)
    Wk = np.asarray(Wk, dtype=np.float32).reshape(DC, HS)
    Wv = np.asarray(Wv, dtype=np.float32).reshape(DC, HS)
    Wo = np.asarray(Wo, dtype=np.float32).reshape(HS, D)
    bo = np.asarray(bo, dtype=np.float32).reshape(D)

    nc = _get_nc(use_f32r=_use_f32r, with_bias=bool(np.any(bo)))
    in_maps = []
    for c in range(N_CORES):
        sl = slice(c * NB, (c + 1) * NB)
        in_maps.append({
            "query": np.ascontiguousarray(query[sl]),
            "context": np.ascontiguousarray(context[sl]),
            "Wq": Wq, "Wk": Wk, "Wv": Wv, "Wo": Wo, "bo": bo,
        })
    res = run_bass_kernel_spmd(nc, in_maps, core_ids=list(range(N_CORES)))
    return np.concatenate([res.results[c]["out"] for c in range(N_CORES)], axis=0)
